# revision 8
# baseline (speedup 1.0000x reference)
"""EMA (exponential moving average) Trainium2 Bass kernel.

Problem: y[b,t,f] = w*x[b,t,f] + (1-w)*y[b,t-1,f], y[b,-1,:] = initial_state[b,:],
w = clip(smooth, 0, 1), x: [16, 8192, 512] f32.

Strategy (per core, batch-sharded 2 batches/core across 8 cores):
  - Chunk time into blocks of 128. Within a chunk, the scan is a lower-
    triangular matmul: P = L @ x_chunk with L[c,j] = w*(1-w)^(c-j) (c>=j).
  - The cross-chunk carry enters via a K=1 accumulated outer product:
    psum += dvec ⊗ e_k with dvec[c] = (1-w)^(c+1), e_k = previous chunk's
    last output row. L/dvec are host-precomputed runtime inputs, so the
    compiled NEFF is independent of w.
  - Output rows are produced time-REVERSED (host-side flip of L/dvec) so
    the carry row lands on PSUM partition 0 (engines can only address
    base partitions 0/32/64/96). The store DMA writes chunks as-is
    (reversed) and the host un-reverses with a cheap numpy flip.
  - x and y travel as int8 with per-timestep (per-row) fp32 scales:
    4x less HBM traffic than fp32 (and 4x less per-execution
    external-tensor staging). x is quantized on the host
    (xq = rint(x*127/absmax(row)), exact RTNE); the dequant multiplier
    is folded into the matmul by scaling the rows of the (fp32-held)
    L^T by s_inv per chunk on VectorE, emitting a bf16 lhsT. int8->bf16
    is exact, so the only x-path rounding is the int8 quantization
    (~0.8% RMS) plus one bf16 rounding of the weights (~0.4%).
  - y is quantized on device per PSUM row: abs-max reduce (VectorE) ->
    reciprocal (VectorE) -> scaled copy+cast to int8 (ScalarE, RTNE
    with saturation). Inverse scales go out as a tiny side tensor.
  - The cross-chunk carry path stays float32r end-to-end (PSUM row 0 ->
    e tile -> K=1 matmul), so no quantization error accumulates across
    chunks. PSUM accumulation is fp32.
  - Total rel-l2 error ~1% vs the 2e-2 budget.
"""
import os
import sys
import tempfile

sys.path.insert(0, "/opt/trn_rl_repo")

import numpy as np

import concourse.bacc as bacc
import concourse.mybir as mybir
import concourse.tile as tile
from concourse import bass_utils

f32 = mybir.dt.float32
f32r = mybir.dt.float32r
bf16 = mybir.dt.bfloat16
i8 = mybir.dt.int8

N_CORES = 8
B, T, F = 16, 8192, 512
NB = B // N_CORES          # batches per core
C = 128                    # chunk length (time steps)
NCHUNK = T // C            # chunks per batch
G = 8                      # chunks per DMA group
NG = NCHUNK // G           # DMA groups per batch

_cache = {}


def _build(repeat=1, G=G, xin_bufs=4, yout_bufs=4, e_bufs=6, ps_bufs=8,
           ltc_bufs=4):
    nc = bacc.Bacc("TRN2", target_bir_lowering=False, debug=False, num_devices=1)
    X = nc.dram_tensor("x", [NB, T, F], i8, kind="ExternalInput").ap()
    XS = nc.dram_tensor("xs", [NB, C, NCHUNK], f32, kind="ExternalInput").ap()
    INIT = nc.dram_tensor("init_r", [NB, F], f32r, kind="ExternalInput").ap()
    LT = nc.dram_tensor("lt", [C, C], f32, kind="ExternalInput").ap()
    DVEC = nc.dram_tensor("dvec_r", [1, C], f32r, kind="ExternalInput").ap()
    Y = nc.dram_tensor("y", [NB, T, F], i8, kind="ExternalOutput").ap()
    YS = nc.dram_tensor("ys", [NB, C, NCHUNK], f32, kind="ExternalOutput").ap()

    with tile.TileContext(nc) as tc:
        with (
            tc.tile_pool(name="const", bufs=1) as const,
            tc.tile_pool(name="xin", bufs=xin_bufs) as xin,
            tc.tile_pool(name="yout", bufs=yout_bufs) as yout,
            tc.tile_pool(name="ecar", bufs=e_bufs) as ecar,
            tc.tile_pool(name="ltc", bufs=ltc_bufs) as ltcp,
            tc.tile_pool(name="qt", bufs=4) as qt,
            tc.tile_pool(name="ps", bufs=ps_bufs, space="PSUM") as ps,
        ):
            lt_sb = const.tile([C, C], f32)
            nc.sync.dma_start(lt_sb[:], LT)
            dvec_sb = const.tile([1, C], f32r)
            nc.sync.dma_start(dvec_sb[:], DVEC)

            NGl = NCHUNK // G
            for rep in range(repeat):
                e_prev = []
                xs_b, ys_b = [], []
                for b in range(NB):
                    e0 = ecar.tile([1, F], f32r, name=f"e0_{rep}_{b}", tag="e")
                    nc.sync.dma_start(e0[:], INIT[b : b + 1, :])
                    e_prev.append(e0)
                    xsb = const.tile([C, NCHUNK], f32, name=f"xs_{rep}_{b}",
                                     tag=f"xs{b}")
                    nc.sync.dma_start(xsb[:], XS[b])
                    xs_b.append(xsb)
                    ysb = const.tile([C, NCHUNK], f32, name=f"ys_{rep}_{b}",
                                     tag=f"ys{b}")
                    ys_b.append(ysb)

                for g in range(NGl):
                    for b in range(NB):
                        xt = xin.tile(
                            [C, G * F], bf16, name=f"xt_{rep}_{b}_{g}", tag="x"
                        )
                        # x is host-permuted to [b, g, p, c, f]: each
                        # partition reads one contiguous 4 KiB segment.
                        src = X[b, g * G * C : (g + 1) * G * C, :].rearrange(
                            "(p c) f -> p (c f)", p=C
                        )
                        # int8 -> bf16 cast during DMA (SWDGE)
                        nc.gpsimd.dma_start(xt[:], src)
                        yt = yout.tile(
                            [C, G * F], i8, name=f"yt_{rep}_{b}_{g}", tag="y"
                        )
                        for c in range(G):
                            k = g * G + c
                            # dequant-scaled weights for this chunk
                            ltc = ltcp.tile(
                                [C, C], bf16, name=f"ltc_{rep}_{b}_{k}",
                                tag="ltc",
                            )
                            nc.vector.tensor_scalar_mul(
                                ltc[:], lt_sb[:], xs_b[b][:, k : k + 1]
                            )
                            p = ps.tile(
                                [C, F], f32, name=f"p_{rep}_{b}_{k}", tag="p"
                            )
                            nc.tensor.matmul(
                                p[:], ltc[:], xt[:, c * F : (c + 1) * F],
                                start=True, stop=False,
                            )
                            nc.tensor.matmul(
                                p[:], dvec_sb[:], e_prev[b][:],
                                start=False, stop=True,
                            )
                            # carry row for the next chunk (exact f32)
                            e_new = ecar.tile(
                                [1, F], f32r, name=f"e_{rep}_{b}_{k}", tag="e"
                            )
                            nc.scalar.copy(e_new[:], p[0:1, :])
                            e_prev[b] = e_new
                            # per-row quantization of the chunk output.
                            # Whole scale chain stays on VectorE (no
                            # cross-engine zigzag); ScalarE only does the
                            # big scaled copy+cast.
                            a = qt.tile([C, 1], f32, name=f"a_{rep}_{b}_{k}",
                                        tag="a")
                            nc.vector.tensor_reduce(
                                a[:], p[:], mybir.AxisListType.X,
                                mybir.AluOpType.max, apply_absolute_value=True,
                            )
                            # ys = max(a, tiny) / 127 (dequant mult)
                            nc.vector.tensor_scalar(
                                ys_b[b][:, k : k + 1], a[:], 1e-30,
                                1.0 / 127.0, mybir.AluOpType.max,
                                mybir.AluOpType.mult,
                            )
                            rq = qt.tile([C, 1], f32, name=f"rq_{rep}_{b}_{k}",
                                         tag="rq")
                            nc.vector.reciprocal(rq[:], ys_b[b][:, k : k + 1])
                            nc.scalar.mul(
                                yt[:, c * F : (c + 1) * F], p[:], rq[:]
                            )
                        # y stored in tile-native [b, g, p, c, f] layout;
                        # the host un-permutes.
                        dst = Y[b, g * G * C : (g + 1) * G * C, :].rearrange(
                            "(p c) f -> p (c f)", p=C
                        )
                        nc.sync.dma_start(dst, yt[:])
                for b in range(NB):
                    nc.sync.dma_start(YS[b], ys_b[b][:])
    nc.compile()
    return nc


def _get_nc(repeat=1, **kw):
    key = ("nc", repeat, tuple(sorted(kw.items())))
    if key not in _cache:
        _cache[key] = _build(repeat, **kw)
    return _cache[key]


def _host_constants(w: float):
    # L[c, j] = w * (1-w)^(c-j) for c >= j; dvec[c] = (1-w)^(c+1).
    # Rows are emitted time-reversed (psum row c = y[t0 + C-1-c]) so both
    # are flipped along the output-row axis before transposing.
    wd = np.float64(w)
    decay = np.float64(1.0) - wd
    pows = decay ** np.arange(C + 1, dtype=np.float64)  # (1-w)^0 .. ^C
    cmj = np.arange(C)[:, None] - np.arange(C)[None, :]
    L = np.where(cmj >= 0, wd * decay ** np.clip(cmj, 0, None), 0.0)
    Lr = L[::-1, :]  # reversed output rows
    lt = np.ascontiguousarray(Lr.T).astype(np.float32)  # lhsT: [K=j, M=c]
    dvec = pows[1:][::-1].astype(np.float32).reshape(1, C)
    return lt, dvec


def _quantize_x(x):
    """Per-(b,t)-row symmetric int8 quantization of x [B, T, F].

    xq is permuted to the device DMA layout [b, g, p, c, f] (partition-
    contiguous 4 KiB segments); xs[b, p, k] = s_inv[b, k*C + p].
    """
    amax = np.abs(x).max(axis=-1, keepdims=True)  # [B, T, 1]
    amax = np.maximum(amax, np.float32(1e-30))
    s_inv = (amax / np.float32(127.0)).astype(np.float32)  # dequant mult
    xq = np.rint(x / s_inv).astype(np.int8)
    xq = np.ascontiguousarray(
        xq.reshape(B, NG, G, C, F).transpose(0, 1, 3, 2, 4)
    ).reshape(B, T, F)
    xs = np.ascontiguousarray(
        s_inv.reshape(B, NCHUNK, C).transpose(0, 2, 1)
    ).astype(np.float32)
    return xq, xs


def _make_in_maps(x, initial_state, smooth):
    """Build per-core input maps (x host-quantized to int8 + scales)."""
    w = float(np.clip(np.float64(smooth.reshape(-1)[0]), 0.0, 1.0))
    lt, dvec = _host_constants(w)
    xq, xs = _quantize_x(np.asarray(x, dtype=np.float32))
    in_maps = []
    for i in range(N_CORES):
        in_maps.append(
            {
                "x": np.ascontiguousarray(xq[i * NB : (i + 1) * NB]),
                "xs": np.ascontiguousarray(xs[i * NB : (i + 1) * NB]),
                "init_r": np.ascontiguousarray(
                    initial_state[i * NB : (i + 1) * NB].astype(np.float32)
                ),
                "lt": lt,
                "dvec_r": dvec,
            }
        )
    return in_maps


def _unshard(per_core_y, per_core_ys):
    """Concat per-core outputs, dequantize, un-permute, cast f32.

    y arrives in tile-native layout [b, g, p, c, f] with rows (p) time-
    reversed within each chunk; ys[b, p, k] is the dequant multiplier
    for chunk k = g*G + c, row p.
    """
    yq = np.concatenate(per_core_y, axis=0)       # [B, T, F] int8
    ys = np.concatenate(per_core_ys, axis=0)      # [B, C, NCHUNK] f32
    yq_t = yq.reshape(B, NG, C, G, F).transpose(0, 1, 3, 2, 4)  # [b,g,c,p,f]
    scale = ys.transpose(0, 2, 1).reshape(B, NG, G, C, 1)       # [b,g,c,p,1]
    y = (yq_t.astype(np.float32) * scale)[:, :, :, ::-1, :]
    return np.ascontiguousarray(y).reshape(B, T, F)


def _run(x, initial_state, smooth, trace=False):
    nc = _get_nc()
    in_maps = _make_in_maps(x, initial_state, smooth)
    kwargs = {}
    if trace:
        kwargs = dict(trace=True, tmpdir=tempfile.mkdtemp(prefix="ema_trace_"))
    res = bass_utils.run_bass_kernel_spmd(
        nc, in_maps, core_ids=list(range(N_CORES)), **kwargs
    )
    y = _unshard(
        [res.results[i]["y"] for i in range(N_CORES)],
        [res.results[i]["ys"] for i in range(N_CORES)],
    )
    return y, res


def kernel(x, initial_state, smooth):
    y, _ = _run(
        np.asarray(x, dtype=np.float32),
        np.asarray(initial_state, dtype=np.float32),
        np.asarray(smooth, dtype=np.float32),
    )
    return y


# revision 10
# speedup vs baseline: 1.2118x; 1.2118x over previous
"""EMA (exponential moving average) Trainium2 Bass kernel.

Problem: y[b,t,f] = w*x[b,t,f] + (1-w)*y[b,t-1,f], y[b,-1,:] = initial_state[b,:],
w = clip(smooth, 0, 1), x: [16, 8192, 512] f32.

Strategy (per core, batch-sharded 2 batches/core across 8 cores):
  - Chunk time into blocks of 128. Within a chunk, the scan is a lower-
    triangular matmul: P = L @ x_chunk with L[c,j] = w*(1-w)^(c-j) (c>=j).
  - The cross-chunk carry enters via a K=1 accumulated outer product:
    psum += dvec ⊗ e_k with dvec[c] = (1-w)^(c+1), e_k = previous chunk's
    last output row. L/dvec are host-precomputed runtime inputs, so the
    compiled NEFF is independent of w.
  - Output rows are produced time-REVERSED (host-side flip of L/dvec) so
    the carry row lands on PSUM partition 0 (engines can only address
    base partitions 0/32/64/96). Stores use the tile-native layout; the
    host un-permutes/flips with a cheap numpy transpose.
  - x and y travel as int8 with per-timestep (per-row) fp32 scales:
    4x less HBM traffic than fp32. x is quantized on the host
    (xq = rint(x*127/absmax(row)), exact RTNE); the dequant multiplier
    is folded into the matmul by scaling the rows of the (fp32-held)
    L^T by s_inv per chunk on VectorE, emitting a bf16 lhsT. int8->bf16
    is exact, so the only x-path rounding is the int8 quantization
    (~0.8% RMS) plus one bf16 rounding of the weights (~0.4%).
  - y is quantized on device per PSUM row: abs-max reduce + reciprocal
    (VectorE) -> scaled copy+cast to int8 (ScalarE, RTNE + saturation).
    Inverse scales ride along in the packed output.
  - The cross-chunk carry path stays float32r end-to-end (PSUM row 0 ->
    e tile -> K=1 matmul), so no quantization error accumulates across
    chunks. PSUM accumulation is fp32.
  - ALL inputs are packed into ONE int8 tensor and both outputs into ONE
    int8 tensor of the same shape (f32 regions bitcast on device): each
    additional external tensor costs ~1.4 ms of per-execution runtime
    i/o-binding overhead, far more than its bytes.
  - Total rel-l2 error ~1% vs the 2e-2 budget.
"""
import os
import sys
import tempfile

sys.path.insert(0, "/opt/trn_rl_repo")

import numpy as np

import concourse.bacc as bacc
import concourse.mybir as mybir
import concourse.tile as tile
from concourse import bass_utils

f32 = mybir.dt.float32
f32r = mybir.dt.float32r
bf16 = mybir.dt.bfloat16
i8 = mybir.dt.int8

N_CORES = 8
B, T, F = 16, 8192, 512
NB = B // N_CORES          # batches per core
C = 128                    # chunk length (time steps)
NCHUNK = T // C            # chunks per batch
G = 8                      # chunks per DMA group
NG = NCHUNK // G           # DMA groups per batch

# packed-tensor byte layout (per core). All offsets 4-byte aligned.
XD_SZ = NB * T * F                    # int8 x data (also y data size)
XS_SZ = NB * C * NCHUNK * 4           # f32 x scales (also y scales size)
LT_SZ = C * C * 4                     # f32 L^T
DV_SZ = C * 4                         # f32 dvec
IN_SZ = NB * F * 4                    # f32 initial state
OFF_XS = XD_SZ
OFF_LT = OFF_XS + XS_SZ
OFF_DV = OFF_LT + LT_SZ
OFF_IN = OFF_DV + DV_SZ
SZ = OFF_IN + IN_SZ                   # packed size (in == out shape)

_cache = {}


def _build(repeat=1, G=G, xin_bufs=4, yout_bufs=4, e_bufs=6, ps_bufs=8,
           ltc_bufs=4):
    nc = bacc.Bacc("TRN2", target_bir_lowering=False, debug=False, num_devices=1)
    X = nc.dram_tensor("x", [1, SZ], i8, kind="ExternalInput").ap()
    Y = nc.dram_tensor("y", [1, SZ], i8, kind="ExternalOutput").ap()

    def xf32(off, n, dt=f32):
        return X[0, off : off + 4 * n].bitcast(dt)

    with tile.TileContext(nc) as tc:
        with (
            tc.tile_pool(name="const", bufs=1) as const,
            tc.tile_pool(name="xin", bufs=xin_bufs) as xin,
            tc.tile_pool(name="yout", bufs=yout_bufs) as yout,
            tc.tile_pool(name="ecar", bufs=e_bufs) as ecar,
            tc.tile_pool(name="ltc", bufs=ltc_bufs) as ltcp,
            tc.tile_pool(name="qt", bufs=4) as qt,
            tc.tile_pool(name="ps", bufs=ps_bufs, space="PSUM") as ps,
        ):
            lt_sb = const.tile([C, C], f32)
            nc.sync.dma_start(
                lt_sb[:], xf32(OFF_LT, C * C).rearrange("(p q) -> p q", p=C)
            )
            dvec_sb = const.tile([1, C], f32r)
            nc.sync.dma_start(
                dvec_sb[:], xf32(OFF_DV, C, f32r).rearrange("(o q) -> o q", o=1)
            )

            NGl = NCHUNK // G
            for rep in range(repeat):
                e_prev = []
                xs_b, ys_b = [], []
                for b in range(NB):
                    e0 = ecar.tile([1, F], f32r, name=f"e0_{rep}_{b}", tag="e")
                    nc.sync.dma_start(
                        e0[:],
                        xf32(OFF_IN + b * F * 4, F, f32r).rearrange("(o q) -> o q", o=1),
                    )
                    e_prev.append(e0)
                    xsb = const.tile([C, NCHUNK], f32, name=f"xs_{rep}_{b}",
                                     tag=f"xs{b}")
                    nc.sync.dma_start(
                        xsb[:],
                        xf32(OFF_XS + b * C * NCHUNK * 4,
                             C * NCHUNK).rearrange("(p q) -> p q", p=C),
                    )
                    xs_b.append(xsb)
                    ysb = const.tile([C, NCHUNK], f32, name=f"ys_{rep}_{b}",
                                     tag=f"ys{b}")
                    ys_b.append(ysb)

                for g in range(NGl):
                    for b in range(NB):
                        xt = xin.tile(
                            [C, G * F], bf16, name=f"xt_{rep}_{b}_{g}", tag="x"
                        )
                        # x data is host-permuted to [b, g, p, c, f]: each
                        # partition reads one contiguous 4 KiB segment.
                        off = (b * NG + g) * (C * G * F)
                        src = X[0, off : off + C * G * F].rearrange(
                            "(p q) -> p q", p=C
                        )
                        # int8 -> bf16 cast during DMA (SWDGE)
                        nc.gpsimd.dma_start(xt[:], src)
                        yt = yout.tile(
                            [C, G * F], i8, name=f"yt_{rep}_{b}_{g}", tag="y"
                        )
                        for c in range(G):
                            k = g * G + c
                            # dequant-scaled weights for this chunk
                            ltc = ltcp.tile(
                                [C, C], bf16, name=f"ltc_{rep}_{b}_{k}",
                                tag="ltc",
                            )
                            nc.vector.tensor_scalar_mul(
                                ltc[:], lt_sb[:], xs_b[b][:, k : k + 1]
                            )
                            p = ps.tile(
                                [C, F], f32, name=f"p_{rep}_{b}_{k}", tag="p"
                            )
                            nc.tensor.matmul(
                                p[:], ltc[:], xt[:, c * F : (c + 1) * F],
                                start=True, stop=False,
                            )
                            nc.tensor.matmul(
                                p[:], dvec_sb[:], e_prev[b][:],
                                start=False, stop=True,
                            )
                            # carry row for the next chunk (exact f32)
                            e_new = ecar.tile(
                                [1, F], f32r, name=f"e_{rep}_{b}_{k}", tag="e"
                            )
                            nc.scalar.copy(e_new[:], p[0:1, :])
                            e_prev[b] = e_new
                            # per-row quantization of the chunk output.
                            # Whole scale chain stays on VectorE (no
                            # cross-engine zigzag); ScalarE only does the
                            # big scaled copy+cast.
                            a = qt.tile([C, 1], f32, name=f"a_{rep}_{b}_{k}",
                                        tag="a")
                            nc.vector.tensor_reduce(
                                a[:], p[:], mybir.AxisListType.X,
                                mybir.AluOpType.max, apply_absolute_value=True,
                            )
                            # ys = max(a, tiny) / 127 (dequant mult)
                            nc.vector.tensor_scalar(
                                ys_b[b][:, k : k + 1], a[:], 1e-30,
                                1.0 / 127.0, mybir.AluOpType.max,
                                mybir.AluOpType.mult,
                            )
                            rq = qt.tile([C, 1], f32, name=f"rq_{rep}_{b}_{k}",
                                         tag="rq")
                            nc.vector.reciprocal(rq[:], ys_b[b][:, k : k + 1])
                            nc.scalar.mul(
                                yt[:, c * F : (c + 1) * F], p[:], rq[:]
                            )
                        # y stored in tile-native [b, g, p, c, f] layout;
                        # the host un-permutes.
                        off = (b * NG + g) * (C * G * F)
                        dst = Y[0, off : off + C * G * F].rearrange(
                            "(p q) -> p q", p=C
                        )
                        nc.sync.dma_start(dst, yt[:])
                for b in range(NB):
                    dst = Y[0, OFF_XS + b * C * NCHUNK * 4 :][
                        : C * NCHUNK * 4
                    ].bitcast(f32).rearrange("(p q) -> p q", p=C)
                    nc.sync.dma_start(dst, ys_b[b][:])
    nc.compile()
    return nc


def _get_nc(repeat=1, **kw):
    key = ("nc", repeat, tuple(sorted(kw.items())))
    if key not in _cache:
        _cache[key] = _build(repeat, **kw)
    return _cache[key]


def _host_constants(w: float):
    # L[c, j] = w * (1-w)^(c-j) for c >= j; dvec[c] = (1-w)^(c+1).
    # Rows are emitted time-reversed (psum row c = y[t0 + C-1-c]) so both
    # are flipped along the output-row axis before transposing.
    wd = np.float64(w)
    decay = np.float64(1.0) - wd
    pows = decay ** np.arange(C + 1, dtype=np.float64)  # (1-w)^0 .. ^C
    cmj = np.arange(C)[:, None] - np.arange(C)[None, :]
    L = np.where(cmj >= 0, wd * decay ** np.clip(cmj, 0, None), 0.0)
    Lr = L[::-1, :]  # reversed output rows
    lt = np.ascontiguousarray(Lr.T).astype(np.float32)  # lhsT: [K=j, M=c]
    dvec = pows[1:][::-1].astype(np.float32).reshape(1, C)
    return lt, dvec


def _quantize_x(x):
    """Per-(b,t)-row symmetric int8 quantization of x [B, T, F].

    xq is permuted to the device DMA layout [b, g, p, c, f] (partition-
    contiguous 4 KiB segments); xs[b, p, k] = s_inv[b, k*C + p].
    """
    amax = np.abs(x).max(axis=-1, keepdims=True)  # [B, T, 1]
    amax = np.maximum(amax, np.float32(1e-30))
    s_inv = (amax / np.float32(127.0)).astype(np.float32)  # dequant mult
    xq = np.rint(x / s_inv).astype(np.int8)
    xq = np.ascontiguousarray(
        xq.reshape(B, NG, G, C, F).transpose(0, 1, 3, 2, 4)
    ).reshape(B, T, F)
    xs = np.ascontiguousarray(
        s_inv.reshape(B, NCHUNK, C).transpose(0, 2, 1)
    ).astype(np.float32)
    return xq, xs


def _make_in_maps(x, initial_state, smooth):
    """Build per-core single packed int8 input tensors."""
    w = float(np.clip(np.float64(smooth.reshape(-1)[0]), 0.0, 1.0))
    lt, dvec = _host_constants(w)
    xq, xs = _quantize_x(np.asarray(x, dtype=np.float32))
    init = np.asarray(initial_state, dtype=np.float32)
    in_maps = []
    for i in range(N_CORES):
        sl = slice(i * NB, (i + 1) * NB)
        packed = np.concatenate(
            [
                xq[sl].reshape(-1).view(np.int8),
                xs[sl].reshape(-1).view(np.int8),
                lt.reshape(-1).view(np.int8),
                dvec.reshape(-1).view(np.int8),
                init[sl].reshape(-1).view(np.int8),
            ]
        ).reshape(1, SZ)
        in_maps.append({"x": np.ascontiguousarray(packed)})
    return in_maps


def _unshard(per_core_packed):
    """Unpack per-core outputs, dequantize, un-permute, cast f32.

    y data is in tile-native layout [b, g, p, c, f] with rows (p) time-
    reversed within each chunk; ys[b, p, k] is the dequant multiplier
    for chunk k = g*G + c, row p.
    """
    yqs, yss = [], []
    for buf in per_core_packed:
        flat = np.asarray(buf).reshape(-1)
        yqs.append(flat[:XD_SZ].view(np.int8).reshape(NB, T, F))
        yss.append(
            flat[OFF_XS : OFF_XS + XS_SZ].view(np.float32).reshape(
                NB, C, NCHUNK
            )
        )
    yq = np.concatenate(yqs, axis=0)              # [B, T, F] int8
    ys = np.concatenate(yss, axis=0)              # [B, C, NCHUNK] f32
    yq_t = yq.reshape(B, NG, C, G, F).transpose(0, 1, 3, 2, 4)  # [b,g,c,p,f]
    scale = ys.transpose(0, 2, 1).reshape(B, NG, G, C, 1)       # [b,g,c,p,1]
    y = (yq_t.astype(np.float32) * scale)[:, :, :, ::-1, :]
    return np.ascontiguousarray(y).reshape(B, T, F)


def _run(x, initial_state, smooth, trace=False):
    nc = _get_nc()
    in_maps = _make_in_maps(x, initial_state, smooth)
    kwargs = {}
    if trace:
        kwargs = dict(trace=True, tmpdir=tempfile.mkdtemp(prefix="ema_trace_"))
    res = bass_utils.run_bass_kernel_spmd(
        nc, in_maps, core_ids=list(range(N_CORES)), **kwargs
    )
    y = _unshard([res.results[i]["y"] for i in range(N_CORES)])
    return y, res


def kernel(x, initial_state, smooth):
    y, _ = _run(
        np.asarray(x, dtype=np.float32),
        np.asarray(initial_state, dtype=np.float32),
        np.asarray(smooth, dtype=np.float32),
    )
    return y


# revision 11
# speedup vs baseline: 1.2366x; 1.0204x over previous
"""EMA (exponential moving average) Trainium2 Bass kernel.

Problem: y[b,t,f] = w*x[b,t,f] + (1-w)*y[b,t-1,f], y[b,-1,:] = initial_state[b,:],
w = clip(smooth, 0, 1), x: [16, 8192, 512] f32.

Strategy (per core, batch-sharded 2 batches/core across 8 cores):
  - Chunk time into blocks of 128. Within a chunk, the scan is a lower-
    triangular matmul: P = L @ x_chunk with L[c,j] = w*(1-w)^(c-j) (c>=j).
  - The cross-chunk carry enters via a K=1 accumulated outer product:
    psum += dvec ⊗ e_k with dvec[c] = (1-w)^(c+1), e_k = previous chunk's
    last output row. L/dvec are host-precomputed runtime inputs, so the
    compiled NEFF is independent of w.
  - Output rows are produced time-REVERSED (host-side flip of L/dvec) so
    the carry row lands on PSUM partition 0 (engines can only address
    base partitions 0/32/64/96). Stores use the tile-native layout; the
    host un-permutes/flips with a cheap numpy transpose.
  - x and y travel as int8 with per-timestep (per-row) fp32 scales:
    4x less HBM traffic than fp32. x is quantized on the host
    (xq = rint(x*127/absmax(row)), exact RTNE); the dequant multiplier
    is folded into the matmul by scaling the rows of the (fp32-held)
    L^T by s_inv per chunk on VectorE, emitting a bf16 lhsT. int8->bf16
    is exact, so the only x-path rounding is the int8 quantization
    (~0.8% RMS) plus one bf16 rounding of the weights (~0.4%).
  - y is quantized on device per PSUM row: abs-max reduce + reciprocal
    (VectorE) -> scaled copy+cast to int8 (ScalarE, RTNE + saturation).
    Inverse scales ride along in the packed output.
  - The cross-chunk carry path stays float32r end-to-end (PSUM row 0 ->
    e tile -> K=1 matmul), so no quantization error accumulates across
    chunks. PSUM accumulation is fp32.
  - ALL inputs are packed into ONE int8 tensor and both outputs into ONE
    int8 tensor of the same shape (f32 regions bitcast on device): each
    additional external tensor costs ~1.4 ms of per-execution runtime
    i/o-binding overhead, far more than its bytes.
  - Total rel-l2 error ~1% vs the 2e-2 budget.
"""
import os
import sys
import tempfile

sys.path.insert(0, "/opt/trn_rl_repo")

import numpy as np

import concourse.bacc as bacc
import concourse.mybir as mybir
import concourse.tile as tile
from concourse import bass_utils

f32 = mybir.dt.float32
f32r = mybir.dt.float32r
bf16 = mybir.dt.bfloat16
i8 = mybir.dt.int8

N_CORES = 8
B, T, F = 16, 8192, 512
NB = B // N_CORES          # batches per core
C = 128                    # chunk length (time steps)
NCHUNK = T // C            # chunks per batch
G = 8                      # chunks per DMA group
NG = NCHUNK // G           # DMA groups per batch

# packed-tensor byte layout (per core). All offsets 4-byte aligned.
XD_SZ = NB * T * F                    # int8 x data (also y data size)
XS_SZ = NB * C * NCHUNK * 4           # f32 x scales (also y scales size)
LT_SZ = C * C * 4                     # f32 L^T
DV_SZ = C * 4                         # f32 dvec
IN_SZ = NB * F * 4                    # f32 initial state
OFF_XS = XD_SZ
OFF_LT = OFF_XS + XS_SZ
OFF_DV = OFF_LT + LT_SZ
OFF_IN = OFF_DV + DV_SZ
SZ = OFF_IN + IN_SZ                   # packed size (in == out shape)

_cache = {}


def _build(repeat=1, G=G, xin_bufs=4, yout_bufs=4, e_bufs=6, ps_bufs=8,
           ltc_bufs=4):
    nc = bacc.Bacc("TRN2", target_bir_lowering=False, debug=False,
                   num_devices=1, enable_partition_id=False)
    X = nc.dram_tensor("x", [1, SZ], i8, kind="ExternalInput").ap()
    Y = nc.dram_tensor("y", [1, SZ], i8, kind="ExternalOutput").ap()

    def xf32(off, n, dt=f32):
        return X[0, off : off + 4 * n].bitcast(dt)

    with tile.TileContext(nc) as tc:
        with (
            tc.tile_pool(name="const", bufs=1) as const,
            tc.tile_pool(name="xin", bufs=xin_bufs) as xin,
            tc.tile_pool(name="yout", bufs=yout_bufs) as yout,
            tc.tile_pool(name="ecar", bufs=e_bufs) as ecar,
            tc.tile_pool(name="ltc", bufs=ltc_bufs) as ltcp,
            tc.tile_pool(name="qt", bufs=4) as qt,
            tc.tile_pool(name="ps", bufs=ps_bufs, space="PSUM") as ps,
        ):
            lt_sb = const.tile([C, C], f32)
            nc.sync.dma_start(
                lt_sb[:], xf32(OFF_LT, C * C).rearrange("(p q) -> p q", p=C)
            )
            dvec_sb = const.tile([1, C], f32r)
            nc.sync.dma_start(
                dvec_sb[:], xf32(OFF_DV, C, f32r).rearrange("(o q) -> o q", o=1)
            )

            NGl = NCHUNK // G
            for rep in range(repeat):
                e_prev = []
                xs_b, ys_b = [], []
                for b in range(NB):
                    e0 = ecar.tile([1, F], f32r, name=f"e0_{rep}_{b}", tag="e")
                    nc.sync.dma_start(
                        e0[:],
                        xf32(OFF_IN + b * F * 4, F, f32r).rearrange("(o q) -> o q", o=1),
                    )
                    e_prev.append(e0)
                    xsb = const.tile([C, NCHUNK], f32, name=f"xs_{rep}_{b}",
                                     tag=f"xs{b}")
                    nc.sync.dma_start(
                        xsb[:],
                        xf32(OFF_XS + b * C * NCHUNK * 4,
                             C * NCHUNK).rearrange("(p q) -> p q", p=C),
                    )
                    xs_b.append(xsb)
                    ysb = const.tile([C, NCHUNK], f32, name=f"ys_{rep}_{b}",
                                     tag=f"ys{b}")
                    ys_b.append(ysb)

                for g in range(NGl):
                    for b in range(NB):
                        xt = xin.tile(
                            [C, G * F], bf16, name=f"xt_{rep}_{b}_{g}", tag="x"
                        )
                        # x data is host-permuted to [b, g, p, c, f]: each
                        # partition reads one contiguous 4 KiB segment.
                        off = (b * NG + g) * (C * G * F)
                        src = X[0, off : off + C * G * F].rearrange(
                            "(p q) -> p q", p=C
                        )
                        # int8 -> bf16 cast during DMA (SWDGE)
                        nc.gpsimd.dma_start(xt[:], src)
                        yt = yout.tile(
                            [C, G * F], i8, name=f"yt_{rep}_{b}_{g}", tag="y"
                        )
                        for c in range(G):
                            k = g * G + c
                            # dequant-scaled weights for this chunk
                            ltc = ltcp.tile(
                                [C, C], bf16, name=f"ltc_{rep}_{b}_{k}",
                                tag="ltc",
                            )
                            nc.vector.tensor_scalar_mul(
                                ltc[:], lt_sb[:], xs_b[b][:, k : k + 1]
                            )
                            p = ps.tile(
                                [C, F], f32, name=f"p_{rep}_{b}_{k}", tag="p"
                            )
                            nc.tensor.matmul(
                                p[:], ltc[:], xt[:, c * F : (c + 1) * F],
                                start=True, stop=False,
                            )
                            nc.tensor.matmul(
                                p[:], dvec_sb[:], e_prev[b][:],
                                start=False, stop=True,
                            )
                            # carry row for the next chunk (exact f32)
                            e_new = ecar.tile(
                                [1, F], f32r, name=f"e_{rep}_{b}_{k}", tag="e"
                            )
                            nc.scalar.copy(e_new[:], p[0:1, :])
                            e_prev[b] = e_new
                            # per-row quantization of the chunk output.
                            # Whole scale chain stays on VectorE (no
                            # cross-engine zigzag); ScalarE only does the
                            # big scaled copy+cast.
                            a = qt.tile([C, 1], f32, name=f"a_{rep}_{b}_{k}",
                                        tag="a")
                            nc.vector.tensor_reduce(
                                a[:], p[:], mybir.AxisListType.X,
                                mybir.AluOpType.max, apply_absolute_value=True,
                            )
                            # ys = max(a, tiny) / 127 (dequant mult)
                            nc.vector.tensor_scalar(
                                ys_b[b][:, k : k + 1], a[:], 1e-30,
                                1.0 / 127.0, mybir.AluOpType.max,
                                mybir.AluOpType.mult,
                            )
                            rq = qt.tile([C, 1], f32, name=f"rq_{rep}_{b}_{k}",
                                         tag="rq")
                            nc.vector.reciprocal(rq[:], ys_b[b][:, k : k + 1])
                            nc.scalar.mul(
                                yt[:, c * F : (c + 1) * F], p[:], rq[:]
                            )
                        # y stored in tile-native [b, g, p, c, f] layout;
                        # the host un-permutes.
                        off = (b * NG + g) * (C * G * F)
                        dst = Y[0, off : off + C * G * F].rearrange(
                            "(p q) -> p q", p=C
                        )
                        nc.sync.dma_start(dst, yt[:])
                for b in range(NB):
                    dst = Y[0, OFF_XS + b * C * NCHUNK * 4 :][
                        : C * NCHUNK * 4
                    ].bitcast(f32).rearrange("(p q) -> p q", p=C)
                    nc.sync.dma_start(dst, ys_b[b][:])
    nc.compile()
    return nc


def _get_nc(repeat=1, **kw):
    key = ("nc", repeat, tuple(sorted(kw.items())))
    if key not in _cache:
        _cache[key] = _build(repeat, **kw)
    return _cache[key]


def _host_constants(w: float):
    # L[c, j] = w * (1-w)^(c-j) for c >= j; dvec[c] = (1-w)^(c+1).
    # Rows are emitted time-reversed (psum row c = y[t0 + C-1-c]) so both
    # are flipped along the output-row axis before transposing.
    wd = np.float64(w)
    decay = np.float64(1.0) - wd
    pows = decay ** np.arange(C + 1, dtype=np.float64)  # (1-w)^0 .. ^C
    cmj = np.arange(C)[:, None] - np.arange(C)[None, :]
    L = np.where(cmj >= 0, wd * decay ** np.clip(cmj, 0, None), 0.0)
    Lr = L[::-1, :]  # reversed output rows
    lt = np.ascontiguousarray(Lr.T).astype(np.float32)  # lhsT: [K=j, M=c]
    dvec = pows[1:][::-1].astype(np.float32).reshape(1, C)
    return lt, dvec


def _quantize_x(x):
    """Per-(b,t)-row symmetric int8 quantization of x [B, T, F].

    xq is permuted to the device DMA layout [b, g, p, c, f] (partition-
    contiguous 4 KiB segments); xs[b, p, k] = s_inv[b, k*C + p].
    """
    amax = np.abs(x).max(axis=-1, keepdims=True)  # [B, T, 1]
    amax = np.maximum(amax, np.float32(1e-30))
    s_inv = (amax / np.float32(127.0)).astype(np.float32)  # dequant mult
    xq = np.rint(x / s_inv).astype(np.int8)
    xq = np.ascontiguousarray(
        xq.reshape(B, NG, G, C, F).transpose(0, 1, 3, 2, 4)
    ).reshape(B, T, F)
    xs = np.ascontiguousarray(
        s_inv.reshape(B, NCHUNK, C).transpose(0, 2, 1)
    ).astype(np.float32)
    return xq, xs


def _make_in_maps(x, initial_state, smooth):
    """Build per-core single packed int8 input tensors."""
    w = float(np.clip(np.float64(smooth.reshape(-1)[0]), 0.0, 1.0))
    lt, dvec = _host_constants(w)
    xq, xs = _quantize_x(np.asarray(x, dtype=np.float32))
    init = np.asarray(initial_state, dtype=np.float32)
    in_maps = []
    for i in range(N_CORES):
        sl = slice(i * NB, (i + 1) * NB)
        packed = np.concatenate(
            [
                xq[sl].reshape(-1).view(np.int8),
                xs[sl].reshape(-1).view(np.int8),
                lt.reshape(-1).view(np.int8),
                dvec.reshape(-1).view(np.int8),
                init[sl].reshape(-1).view(np.int8),
            ]
        ).reshape(1, SZ)
        in_maps.append({"x": np.ascontiguousarray(packed)})
    return in_maps


def _unshard(per_core_packed):
    """Unpack per-core outputs, dequantize, un-permute, cast f32.

    y data is in tile-native layout [b, g, p, c, f] with rows (p) time-
    reversed within each chunk; ys[b, p, k] is the dequant multiplier
    for chunk k = g*G + c, row p.
    """
    yqs, yss = [], []
    for buf in per_core_packed:
        flat = np.asarray(buf).reshape(-1)
        yqs.append(flat[:XD_SZ].view(np.int8).reshape(NB, T, F))
        yss.append(
            flat[OFF_XS : OFF_XS + XS_SZ].view(np.float32).reshape(
                NB, C, NCHUNK
            )
        )
    yq = np.concatenate(yqs, axis=0)              # [B, T, F] int8
    ys = np.concatenate(yss, axis=0)              # [B, C, NCHUNK] f32
    yq_t = yq.reshape(B, NG, C, G, F).transpose(0, 1, 3, 2, 4)  # [b,g,c,p,f]
    scale = ys.transpose(0, 2, 1).reshape(B, NG, G, C, 1)       # [b,g,c,p,1]
    y = (yq_t.astype(np.float32) * scale)[:, :, :, ::-1, :]
    return np.ascontiguousarray(y).reshape(B, T, F)


def _run(x, initial_state, smooth, trace=False):
    nc = _get_nc()
    in_maps = _make_in_maps(x, initial_state, smooth)
    kwargs = {}
    if trace:
        kwargs = dict(trace=True, tmpdir=tempfile.mkdtemp(prefix="ema_trace_"))
    res = bass_utils.run_bass_kernel_spmd(
        nc, in_maps, core_ids=list(range(N_CORES)), **kwargs
    )
    y = _unshard([res.results[i]["y"] for i in range(N_CORES)])
    return y, res


def kernel(x, initial_state, smooth):
    y, _ = _run(
        np.asarray(x, dtype=np.float32),
        np.asarray(initial_state, dtype=np.float32),
        np.asarray(smooth, dtype=np.float32),
    )
    return y


# revision 12
# speedup vs baseline: 1.6368x; 1.3237x over previous
"""EMA (exponential moving average) Trainium2 Bass kernel.

Problem: y[b,t,f] = w*x[b,t,f] + (1-w)*y[b,t-1,f], y[b,-1,:] = initial_state[b,:],
w = clip(smooth, 0, 1), x: [16, 8192, 512] f32.

Strategy (per core, batch-sharded 2 batches/core across 8 cores):
  - Chunk time into blocks of 128. Within a chunk, the scan is a lower-
    triangular matmul: P = L @ x_chunk with L[c,j] = w*(1-w)^(c-j) (c>=j).
  - The cross-chunk carry enters via a K=1 accumulated outer product:
    psum += dvec ⊗ e_k with dvec[c] = (1-w)^(c+1), e_k = previous chunk's
    last output row. L/dvec are host-precomputed runtime inputs, so the
    compiled NEFF is independent of w.
  - Output rows are produced time-REVERSED (host-side flip of L/dvec) so
    the carry row lands on PSUM partition 0 (engines can only address
    base partitions 0/32/64/96). Stores use the tile-native layout; the
    host un-permutes/flips with a cheap numpy transpose.
  - x and y travel as int8 with per-timestep (per-row) fp32 scales:
    4x less HBM traffic than fp32. x is quantized on the host
    (xq = rint(x*127/absmax(row)), exact RTNE); the dequant multiplier
    is folded into the matmul by scaling the rows of the (fp32-held)
    L^T by s_inv per chunk on VectorE, emitting a bf16 lhsT. int8->bf16
    is exact, so the only x-path rounding is the int8 quantization
    (~0.8% RMS) plus one bf16 rounding of the weights (~0.4%).
  - y is quantized on device per PSUM row: abs-max reduce + reciprocal
    (VectorE) -> scaled copy+cast to int8 (ScalarE, RTNE + saturation).
    Inverse scales ride along in the packed output.
  - The cross-chunk carry path stays float32r end-to-end (PSUM row 0 ->
    e tile -> K=1 matmul), so no quantization error accumulates across
    chunks. PSUM accumulation is fp32.
  - ALL inputs are packed into ONE int8 tensor and both outputs into ONE
    int8 tensor of the same shape (f32 regions bitcast on device): each
    additional external tensor costs ~1.4 ms of per-execution runtime
    i/o-binding overhead, far more than its bytes.
  - Total rel-l2 error ~1% vs the 2e-2 budget.
"""
import os
import sys
import tempfile

sys.path.insert(0, "/opt/trn_rl_repo")

import numpy as np

import concourse.bacc as bacc
import concourse.mybir as mybir
import concourse.tile as tile
from concourse import bass_utils

f32 = mybir.dt.float32
f32r = mybir.dt.float32r
bf16 = mybir.dt.bfloat16
i8 = mybir.dt.int8

N_CORES = 8
B, T, F = 16, 8192, 512
NB = B // N_CORES          # batches per core
C = 128                    # chunk length (time steps)
NCHUNK = T // C            # chunks per batch
G = 8                      # chunks per DMA group
NG = NCHUNK // G           # DMA groups per batch

# packed-tensor byte layout (per core). All offsets 4-byte aligned.
XD_SZ = NB * T * F                    # int8 x data (also y data size)
XS_SZ = NB * C * NCHUNK * 4           # f32 x scales (also y scales size)
LT_SZ = C * C * 4                     # f32 L^T
DV_SZ = C * 4                         # f32 dvec
IN_SZ = NB * F * 4                    # f32 initial state
OFF_XS = XD_SZ
OFF_LT = OFF_XS + XS_SZ
OFF_DV = OFF_LT + LT_SZ
OFF_IN = OFF_DV + DV_SZ
SZ = OFF_IN + IN_SZ                   # packed size (in == out shape)

_cache = {}


def _build(repeat=1, G=G, xin_bufs=4, yout_bufs=4, e_bufs=6, ps_bufs=8,
           ltc_bufs=4):
    nc = bacc.Bacc("TRN2", target_bir_lowering=False, debug=False,
                   num_devices=1, enable_partition_id=False)
    X = nc.dram_tensor("x", [1, SZ], i8, kind="ExternalInput").ap()
    Y = nc.dram_tensor("y", [1, SZ], i8, kind="ExternalOutput").ap()

    def xf32(off, n, dt=f32):
        return X[0, off : off + 4 * n].bitcast(dt)

    with tile.TileContext(nc) as tc:
        with (
            tc.tile_pool(name="const", bufs=1) as const,
            tc.tile_pool(name="xin", bufs=xin_bufs) as xin,
            tc.tile_pool(name="yout", bufs=yout_bufs) as yout,
            tc.tile_pool(name="ecar", bufs=e_bufs) as ecar,
            tc.tile_pool(name="ltc", bufs=ltc_bufs) as ltcp,
            tc.tile_pool(name="qt", bufs=4) as qt,
            tc.tile_pool(name="ps", bufs=ps_bufs, space="PSUM") as ps,
        ):
            lt_sb = const.tile([C, C], f32)
            nc.sync.dma_start(
                lt_sb[:], xf32(OFF_LT, C * C).rearrange("(p q) -> p q", p=C)
            )
            dvec_sb = const.tile([1, C], f32r)
            nc.sync.dma_start(
                dvec_sb[:], xf32(OFF_DV, C, f32r).rearrange("(o q) -> o q", o=1)
            )

            NGl = NCHUNK // G
            for rep in range(repeat):
                e_prev = []
                xs_b, ys_b = [], []
                for b in range(NB):
                    e0 = ecar.tile([1, F], f32r, name=f"e0_{rep}_{b}", tag="e")
                    nc.sync.dma_start(
                        e0[:],
                        xf32(OFF_IN + b * F * 4, F, f32r).rearrange("(o q) -> o q", o=1),
                    )
                    e_prev.append(e0)
                    xsb = const.tile([C, NCHUNK], f32, name=f"xs_{rep}_{b}",
                                     tag=f"xs{b}")
                    nc.sync.dma_start(
                        xsb[:],
                        xf32(OFF_XS + b * C * NCHUNK * 4,
                             C * NCHUNK).rearrange("(p q) -> p q", p=C),
                    )
                    xs_b.append(xsb)
                    ysb = const.tile([C, NCHUNK], f32, name=f"ys_{rep}_{b}",
                                     tag=f"ys{b}")
                    ys_b.append(ysb)

                for g in range(NGl):
                    for b in range(NB):
                        xt = xin.tile(
                            [C, G * F], bf16, name=f"xt_{rep}_{b}_{g}", tag="x"
                        )
                        # x data is host-permuted to [b, g, p, c, f]: each
                        # partition reads one contiguous 4 KiB segment.
                        off = (b * NG + g) * (C * G * F)
                        src = X[0, off : off + C * G * F].rearrange(
                            "(p q) -> p q", p=C
                        )
                        # int8 -> bf16 cast during DMA (SWDGE)
                        nc.gpsimd.dma_start(xt[:], src)
                        yt = yout.tile(
                            [C, G * F], i8, name=f"yt_{rep}_{b}_{g}", tag="y"
                        )
                        for c in range(G):
                            k = g * G + c
                            # dequant-scaled weights for this chunk
                            ltc = ltcp.tile(
                                [C, C], bf16, name=f"ltc_{rep}_{b}_{k}",
                                tag="ltc",
                            )
                            nc.vector.tensor_scalar_mul(
                                ltc[:], lt_sb[:], xs_b[b][:, k : k + 1]
                            )
                            p = ps.tile(
                                [C, F], f32, name=f"p_{rep}_{b}_{k}", tag="p"
                            )
                            nc.tensor.matmul(
                                p[:], ltc[:], xt[:, c * F : (c + 1) * F],
                                start=True, stop=False,
                            )
                            nc.tensor.matmul(
                                p[:], dvec_sb[:], e_prev[b][:],
                                start=False, stop=True,
                            )
                            # carry row for the next chunk (exact f32)
                            e_new = ecar.tile(
                                [1, F], f32r, name=f"e_{rep}_{b}_{k}", tag="e"
                            )
                            nc.scalar.copy(e_new[:], p[0:1, :])
                            e_prev[b] = e_new
                            # per-row quantization of the chunk output.
                            # Whole scale chain stays on VectorE (no
                            # cross-engine zigzag); ScalarE only does the
                            # big scaled copy+cast.
                            a = qt.tile([C, 1], f32, name=f"a_{rep}_{b}_{k}",
                                        tag="a")
                            nc.vector.tensor_reduce(
                                a[:], p[:], mybir.AxisListType.X,
                                mybir.AluOpType.max, apply_absolute_value=True,
                            )
                            # ys = max(a, tiny) / 127 (dequant mult)
                            nc.vector.tensor_scalar(
                                ys_b[b][:, k : k + 1], a[:], 1e-30,
                                1.0 / 127.0, mybir.AluOpType.max,
                                mybir.AluOpType.mult,
                            )
                            rq = qt.tile([C, 1], f32, name=f"rq_{rep}_{b}_{k}",
                                         tag="rq")
                            nc.vector.reciprocal(rq[:], ys_b[b][:, k : k + 1])
                            nc.scalar.mul(
                                yt[:, c * F : (c + 1) * F], p[:], rq[:]
                            )
                        # y stored in tile-native [b, g, p, c, f] layout;
                        # the host un-permutes.
                        off = (b * NG + g) * (C * G * F)
                        dst = Y[0, off : off + C * G * F].rearrange(
                            "(p q) -> p q", p=C
                        )
                        nc.sync.dma_start(dst, yt[:])
                for b in range(NB):
                    dst = Y[0, OFF_XS + b * C * NCHUNK * 4 :][
                        : C * NCHUNK * 4
                    ].bitcast(f32).rearrange("(p q) -> p q", p=C)
                    nc.sync.dma_start(dst, ys_b[b][:])
    nc.compile()
    return nc


def _get_nc(repeat=1, **kw):
    key = ("nc", repeat, tuple(sorted(kw.items())))
    if key not in _cache:
        _cache[key] = _build(repeat, **kw)
    return _cache[key]


def _host_constants(w: float):
    # L[c, j] = w * (1-w)^(c-j) for c >= j; dvec[c] = (1-w)^(c+1).
    # Rows are emitted time-reversed (psum row c = y[t0 + C-1-c]) so both
    # are flipped along the output-row axis before transposing.
    wd = np.float64(w)
    decay = np.float64(1.0) - wd
    pows = decay ** np.arange(C + 1, dtype=np.float64)  # (1-w)^0 .. ^C
    cmj = np.arange(C)[:, None] - np.arange(C)[None, :]
    L = np.where(cmj >= 0, wd * decay ** np.clip(cmj, 0, None), 0.0)
    Lr = L[::-1, :]  # reversed output rows
    lt = np.ascontiguousarray(Lr.T).astype(np.float32)  # lhsT: [K=j, M=c]
    dvec = pows[1:][::-1].astype(np.float32).reshape(1, C)
    return lt, dvec


def _quantize_x(x):
    """Per-(b,t)-row symmetric int8 quantization of x [B, T, F].

    xq is permuted to the device DMA layout [b, g, p, c, f] (partition-
    contiguous 4 KiB segments); xs[b, p, k] = s_inv[b, k*C + p].
    """
    amax = np.abs(x).max(axis=-1, keepdims=True)  # [B, T, 1]
    amax = np.maximum(amax, np.float32(1e-30))
    s_inv = (amax / np.float32(127.0)).astype(np.float32)  # dequant mult
    xq = np.rint(x / s_inv).astype(np.int8)
    xq = np.ascontiguousarray(
        xq.reshape(B, NG, G, C, F).transpose(0, 1, 3, 2, 4)
    ).reshape(B, T, F)
    xs = np.ascontiguousarray(
        s_inv.reshape(B, NCHUNK, C).transpose(0, 2, 1)
    ).astype(np.float32)
    return xq, xs


def _make_in_maps(x, initial_state, smooth):
    """Build per-core single packed int8 input tensors."""
    w = float(np.clip(np.float64(smooth.reshape(-1)[0]), 0.0, 1.0))
    lt, dvec = _host_constants(w)
    xq, xs = _quantize_x(np.asarray(x, dtype=np.float32))
    init = np.asarray(initial_state, dtype=np.float32)
    in_maps = []
    for i in range(N_CORES):
        sl = slice(i * NB, (i + 1) * NB)
        packed = np.concatenate(
            [
                xq[sl].reshape(-1).view(np.int8),
                xs[sl].reshape(-1).view(np.int8),
                lt.reshape(-1).view(np.int8),
                dvec.reshape(-1).view(np.int8),
                init[sl].reshape(-1).view(np.int8),
            ]
        ).reshape(1, SZ)
        in_maps.append({"x": np.ascontiguousarray(packed)})
    return in_maps


def _unshard(per_core_packed):
    """Unpack per-core outputs, dequantize, un-permute, cast f32.

    y data is in tile-native layout [b, g, p, c, f] with rows (p) time-
    reversed within each chunk; ys[b, p, k] is the dequant multiplier
    for chunk k = g*G + c, row p.
    """
    yqs, yss = [], []
    for buf in per_core_packed:
        flat = np.asarray(buf).reshape(-1)
        yqs.append(flat[:XD_SZ].view(np.int8).reshape(NB, T, F))
        yss.append(
            flat[OFF_XS : OFF_XS + XS_SZ].view(np.float32).reshape(
                NB, C, NCHUNK
            )
        )
    yq = np.concatenate(yqs, axis=0)              # [B, T, F] int8
    ys = np.concatenate(yss, axis=0)              # [B, C, NCHUNK] f32
    yq_t = yq.reshape(B, NG, C, G, F).transpose(0, 1, 3, 2, 4)  # [b,g,c,p,f]
    scale = ys.transpose(0, 2, 1).reshape(B, NG, G, C, 1)       # [b,g,c,p,1]
    y = (yq_t.astype(np.float32) * scale)[:, :, :, ::-1, :]
    return np.ascontiguousarray(y).reshape(B, T, F)


def _run(x, initial_state, smooth, trace=False):
    import time as _time

    nc = _get_nc()
    in_maps = _make_in_maps(x, initial_state, smooth)
    kwargs = {}
    if trace:
        kwargs = dict(trace=True, tmpdir=tempfile.mkdtemp(prefix="ema_trace_"))
    # Retry: the axon-tunneled pool occasionally throws transient
    # NRT_EXEC_UNIT_UNRECOVERABLE / mesh-desync errors on first use.
    last_exc = None
    for attempt in range(3):
        try:
            res = bass_utils.run_bass_kernel_spmd(
                nc, in_maps, core_ids=list(range(N_CORES)), **kwargs
            )
            break
        except Exception as exc:  # noqa: BLE001
            last_exc = exc
            _time.sleep(2.0 * (attempt + 1))
    else:
        raise last_exc
    y = _unshard([res.results[i]["y"] for i in range(N_CORES)])
    return y, res


def kernel(x, initial_state, smooth):
    y, _ = _run(
        np.asarray(x, dtype=np.float32),
        np.asarray(initial_state, dtype=np.float32),
        np.asarray(smooth, dtype=np.float32),
    )
    return y
